# revision 22
# baseline (speedup 1.0000x reference)
"""Trainium2 Bass kernel for nn_BTT: out = x.reshape(-1,4096) @ G + bias,
where G (4096x4096) is materialized from three small tensor-train cores.

Strategy:
  - Host: build G from the TT cores (~0.4 GFLOP, 0.15% of total work),
    pre-tile/transpose operands for ideal DMA layout.
  - Device (8 NeuronCores, data-parallel over the 8192-row batch):
    each core computes outT[4096, 1024] = G^T-contraction against its
    1024-row x shard via PE matmuls with G tiles as the stationary
    operand (streamed from HBM once) and x resident in SBUF.
    Bias is fused into the PSUM->SBUF drain on the Scalar engine.

self-contained: hardcodes all shapes; no sibling imports.
"""

import numpy as np

D = 16
R = 8
SIZE = 4096          # D**3
B0, B1 = 8, 1024     # x: (B0, B1, SIZE); total rows = 8192
N_CORES = 8
M = 1024             # batch rows per core
KT = 32              # k tiles of 128 (contraction dim SIZE)
NT = 32              # n tiles of 128 (output cols on PSUM partitions)
NL = 2               # n tiles per group
NG = NT // NL        # 16 groups
MT = 2               # moving-dim tiles of 512 (rows of x shard)
KF = 4               # k tiles fetched per G DMA

# Precision mode for the PE matmuls:
#   "f32"   - native fp32 (4 cycles/row, bit-faithful baseline)
#   "f32r"  - float32r fast fp32 path (1 cycle/row; precision TBD on HW)
#   "f16x3" - fp16 hi/lo split, 3 passes (near-fp32 accuracy, 3 cycles/row)
#   "f16"   - single fp16 pass (1 cycle/row, ~1e-3 relative error)
#   "bf16"  - single bf16 pass (1 cycle/row, ~1e-2 relative error)
MODE = "f16x3"
TRACE = False        # set True from test.py to profile

_prog_cache = {}


def _build_G(core0, core1, core2):
    """G[(j,i1,i2),(y,x,z)] = sum_{b1,b2} core0[r,y,b1]*core1[r,x,b2,b1]*core2[r,z,b2]
    with r the flattened row triple. Mirrors reference.to_matrix contraction order."""
    c0 = np.asarray(core0, np.float32).reshape(SIZE, D, R)       # r, y, b1
    c1 = np.asarray(core1, np.float32).reshape(SIZE, D, R, R)    # r, x, b2, b1
    c2 = np.asarray(core2, np.float32).reshape(SIZE, D, R)       # r, z, b2
    t = np.einsum("rxcb,ryb->ryxc", c1, c0)                      # r, y, x, b2
    G = np.einsum("rzc,ryxc->ryxz", c2, t)                       # r, y, x, z
    return np.ascontiguousarray(G.reshape(SIZE, SIZE))


def _split_f16(a):
    hi = a.astype(np.float16)
    lo = (a - hi.astype(np.float32)).astype(np.float16)
    return hi, lo


def _build_program(mode):
    import concourse.bass as bass
    import concourse.mybir as mybir
    import concourse.tile as tile
    from concourse import bacc
    from contextlib import ExitStack

    f32 = mybir.dt.float32
    if mode == "f32":
        mm_dt = f32
    elif mode == "f32r":
        mm_dt = mybir.dt.float32r
    elif mode in ("f16", "f16x3"):
        mm_dt = mybir.dt.float16
    elif mode == "bf16":
        mm_dt = mybir.dt.bfloat16
    else:
        raise ValueError(mode)
    n_planes = 2 if mode == "f16x3" else 1

    # Bacc: its compile() runs the wait-legalization passes
    # (move_matmul_waits_to_ldweights, generate_event_semaphores) that the
    # TRN2 ISA's 1-wait-per-instruction limit requires.
    nc = bacc.Bacc(None)

    # DRAM I/O (per-core shapes). Host pre-tiles everything so every DMA
    # is a plain contiguous block.
    #   x planes:  [KT, 128, M]     (k-tile major, partitions = k within tile)
    #   G planes:  [NG, KT, 128, NL*128]
    #   biasP:     [128, NT]        (partition-major per n-tile)
    #   outT:      [NT, 128, M]
    xs = [
        nc.dram_tensor(f"x{i}", [KT, 128, M], mm_dt, kind="ExternalInput")
        for i in range(n_planes)
    ]
    # G pre-tiled on host so the device fetch is a plain 2D DMA:
    # g[ng, kc, p, ki*C + c] with C = NL*128 cols per group, KF k-tiles/chunk
    gs = [
        nc.dram_tensor(
            f"g{i}", [NG, KT // KF, 128, KF * NL * 128], mm_dt, kind="ExternalInput"
        )
        for i in range(n_planes)
    ]
    biasP = nc.dram_tensor("biasP", [128, NT], f32, kind="ExternalInput")
    outT = nc.dram_tensor("outT", [NT, 128, M], f32, kind="ExternalOutput")

    with ExitStack() as ctx:
        tc = ctx.enter_context(tile.TileContext(nc))
        xpool = ctx.enter_context(tc.tile_pool(name="x", bufs=KT * n_planes))
        gpool = ctx.enter_context(tc.tile_pool(name="g", bufs=8))
        bpool = ctx.enter_context(tc.tile_pool(name="bias", bufs=1))
        opool = ctx.enter_context(tc.tile_pool(name="out", bufs=4))
        pspool = ctx.enter_context(tc.tile_pool(name="psum", bufs=8, space="PSUM"))

        bias_sb = bpool.tile([128, NT], f32)
        nc.sync.dma_start(bias_sb[:], biasP[:])

        # x resident in SBUF: per k-tile, per plane. Loaded lazily (interleaved
        # with the first group's G fetches) so PE can start ~immediately.
        x_sb = [[None] * KT for _ in range(n_planes)]

        def load_x(kt):
            if x_sb[0][kt] is None:
                for pl in range(n_planes):
                    t = xpool.tile([128, M], mm_dt, name=f"x{pl}_{kt}", tag="x")
                    nc.sync.dma_start(t[:], xs[pl][kt])
                    x_sb[pl][kt] = t

        for ng in range(NG):
            psums = [
                [
                    pspool.tile([128, 512], f32, name=f"ps{nl}_{mt}", tag="ps")
                    for mt in range(MT)
                ]
                for nl in range(NL)
            ]
            for kt0 in range(0, KT, KF):
                for kt in range(kt0, kt0 + KF):
                    load_x(kt)
                g4 = [
                    gpool.tile(
                        [128, KF * NL * 128], mm_dt, name=f"g{pl}", tag=f"g{pl}"
                    )
                    for pl in range(n_planes)
                ]
                for pl in range(n_planes):
                    # gpsimd (SWDGE): slot-recycle WAW/WAR deps need >1 wait,
                    # which the HWDGE direct-2D DMA instruction can't carry.
                    nc.gpsimd.dma_start(g4[pl][:], gs[pl][ng, kt0 // KF])
                for ki in range(KF):
                    kt = kt0 + ki
                    start = kt == 0
                    stop = kt == KT - 1
                    # passes: (x_hi,g_hi), (x_hi,g_lo), then (x_lo,g_hi) last —
                    # x_hi-only first so the x_lo DMAs get arrival slack
                    # during the first group's cold-start streaming.
                    if n_planes == 2:
                        phases = [(0, 0), (1, 0), (0, 1)]
                    else:
                        phases = [(0, 0)]
                    for nl in range(NL):
                        for pi, (pl_g, pl_x) in enumerate(phases):
                            base = ki * NL * 128 + nl * 128
                            lhsT = g4[pl_g][:, base : base + 128]
                            first = start and pi == 0
                            last = stop and pi == len(phases) - 1
                            for mt in range(MT):
                                nc.tensor.matmul(
                                    psums[nl][mt][:],
                                    lhsT,
                                    x_sb[pl_x][kt][:, mt * 512 : (mt + 1) * 512],
                                    start=first,
                                    stop=last,
                                )
            for nl in range(NL):
                nt = ng * NL + nl
                o = opool.tile([128, M], f32, name="o", tag="o")
                for mt in range(MT):
                    nc.scalar.activation(
                        o[:, mt * 512 : (mt + 1) * 512],
                        psums[nl][mt][:],
                        mybir.ActivationFunctionType.Identity,
                        bias=bias_sb[:, nt : nt + 1],
                    )
                nc.sync.dma_start(outT[nt], o[:])

    nc.compile()
    return nc


def _get_program(mode):
    if mode not in _prog_cache:
        _prog_cache[mode] = _build_program(mode)
    return _prog_cache[mode]


def _prep_inputs(x, core0, core1, core2, bias, mode):
    """Host-side shard + layout prep. Returns in_maps for 8 cores."""
    G = _build_G(core0, core1, core2)
    x = np.asarray(x, np.float32)

    # G tiled for 2D DMA: [NG, KT//KF, 128, KF*NL*128]
    # g[ng, kc, p, ki*C + c] = G[(kc*KF+ki)*128 + p, ng*C + c],  C = NL*128
    C = NL * 128
    Gt = np.ascontiguousarray(
        G.reshape(KT // KF, KF, 128, NG, C).transpose(3, 0, 2, 1, 4)
    ).reshape(NG, KT // KF, 128, KF * C)
    biasP = np.ascontiguousarray(
        np.asarray(bias, np.float32).reshape(NT, 128).T
    )

    if mode == "f16x3":
        g_planes = _split_f16(Gt)
    elif mode in ("f16", "bf16"):
        dt = np.float16 if mode == "f16" else None
        if mode == "bf16":
            import ml_dtypes

            dt = ml_dtypes.bfloat16
        g_planes = (Gt.astype(dt),)
    else:
        g_planes = (Gt,)

    in_maps = []
    for c in range(N_CORES):
        xT = np.ascontiguousarray(x[c].T).reshape(KT, 128, M)
        if mode == "f16x3":
            x_planes = _split_f16(xT)
        elif mode in ("f16", "bf16"):
            x_planes = (xT.astype(g_planes[0].dtype),)
        else:
            x_planes = (xT,)
        m = {"biasP": biasP}
        for i, p in enumerate(x_planes):
            m[f"x{i}"] = p
        for i, p in enumerate(g_planes):
            m[f"g{i}"] = p
        in_maps.append(m)
    return in_maps


_last_exec_ns = None


def _ensure_axon_hooks():
    """run_bass_kernel_spmd(trace=True) under axon imports antenv.axon_hooks,
    which is absent from some agent images. Install a best-effort shim so a
    trace request degrades gracefully instead of crashing."""
    try:
        import antenv.axon_hooks  # noqa: F401

        return
    except ImportError:
        pass
    try:
        import sys
        import types

        import antenv

        mod = types.ModuleType("antenv.axon_hooks")
        _h = [None]
        mod.set_axon_ntff_profile_hook = lambda h: _h.__setitem__(0, h)
        mod.get_axon_ntff_profile_hook = lambda: _h[0]
        sys.modules["antenv.axon_hooks"] = mod
        antenv.axon_hooks = mod
        try:
            from trn_agent_boot.trn_boot import _ntff_profile_via_ctypes

            hook = _ntff_profile_via_ctypes("/opt/axon/libaxon_pjrt.so")
            if hook is not None:
                mod.set_axon_ntff_profile_hook(hook)
        except Exception:
            pass
    except Exception:
        pass


def kernel(x, core0, core1, core2, bias):
    global _last_exec_ns
    from concourse.bass_utils import run_bass_kernel_spmd

    _ensure_axon_hooks()

    mode = MODE
    nc = _get_program(mode)
    in_maps = _prep_inputs(x, core0, core1, core2, bias, mode)
    res = run_bass_kernel_spmd(
        nc, in_maps, core_ids=list(range(N_CORES)), trace=TRACE
    )
    _last_exec_ns = res.exec_time_ns
    out = np.stack(
        [r["outT"].transpose(2, 0, 1).reshape(M, SIZE) for r in res.results]
    )
    return out.astype(np.float32)


# revision 25
# speedup vs baseline: 2.6334x; 2.6334x over previous
"""Trainium2 Bass kernel for nn_BTT: out = x.reshape(-1,4096) @ G + bias,
where G (4096x4096) is materialized from three small tensor-train cores.

Strategy:
  - Host: build G from the TT cores (~0.4 GFLOP, 0.15% of total work),
    pre-tile/transpose operands for ideal DMA layout.
  - Device (8 NeuronCores, data-parallel over the 8192-row batch):
    each core computes outT[4096, 1024] = G^T-contraction against its
    1024-row x shard via PE matmuls with G tiles as the stationary
    operand (streamed from HBM once) and x resident in SBUF.
    Bias is fused into the PSUM->SBUF drain on the Scalar engine.

self-contained: hardcodes all shapes; no sibling imports.
"""

import numpy as np

D = 16
R = 8
SIZE = 4096          # D**3
B0, B1 = 8, 1024     # x: (B0, B1, SIZE); total rows = 8192
N_CORES = 8
M = 1024             # batch rows per core
KT = 32              # k tiles of 128 (contraction dim SIZE)
NT = 32              # n tiles of 128 (output cols on PSUM partitions)
NL = 2               # n tiles per group
NG = NT // NL        # 16 groups
MT = 2               # moving-dim tiles of 512 (rows of x shard)
KF = 4               # k tiles fetched per G DMA

# Precision mode for the PE matmuls:
#   "f32"   - native fp32 (4 cycles/row, bit-faithful baseline)
#   "f32r"  - float32r fast fp32 path (1 cycle/row; precision TBD on HW)
#   "f16x3" - fp16 hi/lo split, 3 passes (near-fp32 accuracy, 3 cycles/row)
#   "f16"   - single fp16 pass (1 cycle/row, ~1e-3 relative error)
#   "bf16"  - single bf16 pass (1 cycle/row, ~1e-2 relative error)
MODE = "f16x3"
TRACE = False        # set True from test.py to profile

_prog_cache = {}


def _build_G(core0, core1, core2):
    """G[(j,i1,i2),(y,x,z)] = sum_{b1,b2} core0[r,y,b1]*core1[r,x,b2,b1]*core2[r,z,b2]
    with r the flattened row triple. Mirrors reference.to_matrix contraction order."""
    c0 = np.asarray(core0, np.float32).reshape(SIZE, D, R)       # r, y, b1
    c1 = np.asarray(core1, np.float32).reshape(SIZE, D, R, R)    # r, x, b2, b1
    c2 = np.asarray(core2, np.float32).reshape(SIZE, D, R)       # r, z, b2
    t = np.einsum("rxcb,ryb->ryxc", c1, c0)                      # r, y, x, b2
    G = np.einsum("rzc,ryxc->ryxz", c2, t)                       # r, y, x, z
    return np.ascontiguousarray(G.reshape(SIZE, SIZE))


def _split_f16(a):
    hi = a.astype(np.float16)
    lo = (a - hi.astype(np.float32)).astype(np.float16)
    return hi, lo


def _round13(a):
    """Round fp32 to the 13-bit-mantissa grid (RN). float32r TRUNCATES the low
    10 mantissa bits in the PE; pre-rounding on host removes the truncation
    bias so the hardware truncation becomes exact."""
    u = np.ascontiguousarray(a, np.float32).view(np.uint32)
    return ((u + 0x200) & np.uint32(0xFFFFFC00)).view(np.float32)


def _build_program(mode):
    import concourse.bass as bass
    import concourse.mybir as mybir
    import concourse.tile as tile
    from concourse import bacc
    from contextlib import ExitStack

    f32 = mybir.dt.float32
    if mode == "f32":
        mm_dt = f32
    elif mode == "f32r":
        mm_dt = mybir.dt.float32r
    elif mode in ("f16", "f16x3"):
        mm_dt = mybir.dt.float16
    elif mode == "bf16":
        mm_dt = mybir.dt.bfloat16
    else:
        raise ValueError(mode)
    n_planes = 2 if mode == "f16x3" else 1

    # Bacc: its compile() runs the wait-legalization passes
    # (move_matmul_waits_to_ldweights, generate_event_semaphores) that the
    # TRN2 ISA's 1-wait-per-instruction limit requires.
    nc = bacc.Bacc(None)

    # DRAM I/O (per-core shapes). Host pre-tiles everything so every DMA
    # is a plain contiguous block.
    #   x planes:  [KT, 128, M]     (k-tile major, partitions = k within tile)
    #   G planes:  [NG, KT, 128, NL*128]
    #   biasP:     [128, NT]        (partition-major per n-tile)
    #   outT:      [NT, 128, M]
    xs = [
        nc.dram_tensor(f"x{i}", [KT, 128, M], mm_dt, kind="ExternalInput")
        for i in range(n_planes)
    ]
    # G pre-tiled on host so the device fetch is a plain 2D DMA:
    # g[ng, kc, p, ki*C + c] with C = NL*128 cols per group, KF k-tiles/chunk
    gs = [
        nc.dram_tensor(
            f"g{i}", [NG, KT // KF, 128, KF * NL * 128], mm_dt, kind="ExternalInput"
        )
        for i in range(n_planes)
    ]
    biasP = nc.dram_tensor("biasP", [128, NT], f32, kind="ExternalInput")
    outT = nc.dram_tensor("outT", [NT, 128, M], f32, kind="ExternalOutput")

    with ExitStack() as ctx:
        tc = ctx.enter_context(tile.TileContext(nc))
        xpool = ctx.enter_context(tc.tile_pool(name="x", bufs=KT * n_planes))
        gpool = ctx.enter_context(tc.tile_pool(name="g", bufs=8))
        bpool = ctx.enter_context(tc.tile_pool(name="bias", bufs=1))
        opool = ctx.enter_context(tc.tile_pool(name="out", bufs=4))
        pspool = ctx.enter_context(tc.tile_pool(name="psum", bufs=8, space="PSUM"))

        bias_sb = bpool.tile([128, NT], f32)
        nc.sync.dma_start(bias_sb[:], biasP[:])

        # x resident in SBUF: per k-tile, per plane. Loaded lazily (interleaved
        # with the first group's G fetches) so PE can start ~immediately.
        x_sb = [[None] * KT for _ in range(n_planes)]

        def load_x(kt):
            if x_sb[0][kt] is None:
                for pl in range(n_planes):
                    t = xpool.tile([128, M], mm_dt, name=f"x{pl}_{kt}", tag="x")
                    nc.sync.dma_start(t[:], xs[pl][kt])
                    x_sb[pl][kt] = t

        for ng in range(NG):
            psums = [
                [
                    pspool.tile([128, 512], f32, name=f"ps{nl}_{mt}", tag="ps")
                    for mt in range(MT)
                ]
                for nl in range(NL)
            ]
            for kt0 in range(0, KT, KF):
                for kt in range(kt0, kt0 + KF):
                    load_x(kt)
                g4 = [
                    gpool.tile(
                        [128, KF * NL * 128], mm_dt, name=f"g{pl}", tag=f"g{pl}"
                    )
                    for pl in range(n_planes)
                ]
                for pl in range(n_planes):
                    # gpsimd (SWDGE): slot-recycle WAW/WAR deps need >1 wait,
                    # which the HWDGE direct-2D DMA instruction can't carry.
                    nc.gpsimd.dma_start(g4[pl][:], gs[pl][ng, kt0 // KF])
                for ki in range(KF):
                    kt = kt0 + ki
                    start = kt == 0
                    stop = kt == KT - 1
                    # passes: (x_hi,g_hi), (x_hi,g_lo), then (x_lo,g_hi) last —
                    # x_hi-only first so the x_lo DMAs get arrival slack
                    # during the first group's cold-start streaming.
                    if n_planes == 2:
                        phases = [(0, 0), (1, 0), (0, 1)]
                    else:
                        phases = [(0, 0)]
                    for nl in range(NL):
                        for pi, (pl_g, pl_x) in enumerate(phases):
                            base = ki * NL * 128 + nl * 128
                            lhsT = g4[pl_g][:, base : base + 128]
                            first = start and pi == 0
                            last = stop and pi == len(phases) - 1
                            for mt in range(MT):
                                nc.tensor.matmul(
                                    psums[nl][mt][:],
                                    lhsT,
                                    x_sb[pl_x][kt][:, mt * 512 : (mt + 1) * 512],
                                    start=first,
                                    stop=last,
                                )
            for nl in range(NL):
                nt = ng * NL + nl
                o = opool.tile([128, M], f32, name="o", tag="o")
                for mt in range(MT):
                    nc.scalar.activation(
                        o[:, mt * 512 : (mt + 1) * 512],
                        psums[nl][mt][:],
                        mybir.ActivationFunctionType.Identity,
                        bias=bias_sb[:, nt : nt + 1],
                    )
                nc.sync.dma_start(outT[nt], o[:])

    nc.compile()
    return nc


def _get_program(mode):
    if mode not in _prog_cache:
        _prog_cache[mode] = _build_program(mode)
    return _prog_cache[mode]


def _prep_inputs(x, core0, core1, core2, bias, mode):
    """Host-side shard + layout prep. Returns in_maps for 8 cores."""
    G = _build_G(core0, core1, core2)
    x = np.asarray(x, np.float32)

    # G tiled for 2D DMA: [NG, KT//KF, 128, KF*NL*128]
    # g[ng, kc, p, ki*C + c] = G[(kc*KF+ki)*128 + p, ng*C + c],  C = NL*128
    C = NL * 128
    Gt = np.ascontiguousarray(
        G.reshape(KT // KF, KF, 128, NG, C).transpose(3, 0, 2, 1, 4)
    ).reshape(NG, KT // KF, 128, KF * C)
    biasP = np.ascontiguousarray(
        np.asarray(bias, np.float32).reshape(NT, 128).T
    )

    if mode == "f16x3":
        g_planes = _split_f16(Gt)
    elif mode in ("f16", "bf16"):
        dt = np.float16 if mode == "f16" else None
        if mode == "bf16":
            import ml_dtypes

            dt = ml_dtypes.bfloat16
        g_planes = (Gt.astype(dt),)
    elif mode == "f32r":
        g_planes = (_round13(Gt),)
    else:
        g_planes = (Gt,)

    in_maps = []
    for c in range(N_CORES):
        xT = np.ascontiguousarray(x[c].T).reshape(KT, 128, M)
        if mode == "f16x3":
            x_planes = _split_f16(xT)
        elif mode in ("f16", "bf16"):
            x_planes = (xT.astype(g_planes[0].dtype),)
        elif mode == "f32r":
            x_planes = (_round13(xT),)
        else:
            x_planes = (xT,)
        m = {"biasP": biasP}
        for i, p in enumerate(x_planes):
            m[f"x{i}"] = p
        for i, p in enumerate(g_planes):
            m[f"g{i}"] = p
        in_maps.append(m)
    return in_maps


_last_exec_ns = None


def _ensure_axon_hooks():
    """run_bass_kernel_spmd(trace=True) under axon imports antenv.axon_hooks,
    which is absent from some agent images. Install a best-effort shim so a
    trace request degrades gracefully instead of crashing."""
    try:
        import antenv.axon_hooks  # noqa: F401

        return
    except ImportError:
        pass
    try:
        import sys
        import types

        import antenv

        mod = types.ModuleType("antenv.axon_hooks")
        _h = [None]
        mod.set_axon_ntff_profile_hook = lambda h: _h.__setitem__(0, h)
        mod.get_axon_ntff_profile_hook = lambda: _h[0]
        sys.modules["antenv.axon_hooks"] = mod
        antenv.axon_hooks = mod
        try:
            from trn_agent_boot.trn_boot import _ntff_profile_via_ctypes

            hook = _ntff_profile_via_ctypes("/opt/axon/libaxon_pjrt.so")
            if hook is not None:
                mod.set_axon_ntff_profile_hook(hook)
        except Exception:
            pass
    except Exception:
        pass


def kernel(x, core0, core1, core2, bias):
    global _last_exec_ns
    from concourse.bass_utils import run_bass_kernel_spmd

    _ensure_axon_hooks()

    mode = MODE
    nc = _get_program(mode)
    in_maps = _prep_inputs(x, core0, core1, core2, bias, mode)
    res = run_bass_kernel_spmd(
        nc, in_maps, core_ids=list(range(N_CORES)), trace=TRACE
    )
    _last_exec_ns = res.exec_time_ns
    out = np.stack(
        [r["outT"].transpose(2, 0, 1).reshape(M, SIZE) for r in res.results]
    )
    return out.astype(np.float32)
